# revision 8
# baseline (speedup 1.0000x reference)
"""Trainium2 Bass kernel for the KernelScDM problem (8-core SPMD).

Computes, for X (N,16) and Xref (M,16) with N=M=8192:
  W0    = exp(-||x_i - xref_j||^2 / (4 eps))          (N,M)
  Dref  = rowsum(rbf(Xref,Xref))^-t                   (M,)
  Dinv1ref = (Dref * (Wr@Dref))^-0.5                  (M,)
  Dx    = rowsum(W0)^-t ; Dinv1x = (Dx * (W0@Dref))^-0.5
  W     = Dinv1x[:,None]*Dx[:,None] * W0 * Dref[None,:]*Dinv1ref[None,:]

Sharding: rows of X (and of the Xref x Xref reference matrix) split
across 8 cores; Dref / Dinv1ref shards exchanged with two AllGathers.

The -s*d2 kernel argument is produced on the PE as one matmul over
augmented inputs, with fp32 accuracy recovered from bf16 operands via a
hi/lo split (a.b ~= ah.bh + ah.bl + al.bh). exp runs on ACT with fused
row-sum accumulation; the Dref-weighted row-sum and the final scaling
run as single fused scalar_tensor_tensor ops on the DVE.

Wall-clock here is dominated by the axon tunnel (~25-45 MB/s), not the
device: the result matrix crosses the wire once down (plus a same-size
zero-donation staging cost up, imposed by run_bass_via_pjrt). The
device therefore emits W in bf16 — halving both leg costs vs fp32 —
and the host upcasts to fp32 during the unshard. bf16 keeps elementwise
error ~4e-3, comfortably inside the 2e-2 gate; fp8 (~6% elementwise)
would not pass, and sub-16-bit packing loses its wire savings to
host-side decode.
"""

import json

import numpy as np
import ml_dtypes

import concourse.bass as bass
import concourse.mybir as mybir
from concourse.tile import TileContext
from concourse.bass_utils import run_bass_kernel_spmd

F32 = mybir.dt.float32
BF16 = mybir.dt.bfloat16
AF = mybir.ActivationFunctionType
OP = mybir.AluOpType

N = 8192
M = 8192
D = 16
NCORES = 8
SH = N // NCORES          # rows per core
P = 128                   # partitions
NST = SH // P             # stripes per core (8)
CB = 2048                 # column block (psum tile width)
NCB = M // CB             # column blocks (4)
MMW = 512                 # single-matmul moving width
KXY = 3 * D               # hi/lo split-K rows for the dot product (48)
KZ = KXY + 2              # + norm-term hi/lo rows (50)
KFULL = KZ + 2            # + lnDref hi/lo rows, phase B only (52)


def _softplus(x):
    x = np.float32(x)
    return np.float32(np.log1p(np.exp(-abs(x))) + max(x, 0.0))


def _hilo(v):
    """Split fp32 array into (hi, lo) bf16 parts; hi+lo ~ v to ~16 mantissa bits."""
    hi = v.astype(ml_dtypes.bfloat16)
    lo = (v - hi.astype(np.float32)).astype(ml_dtypes.bfloat16)
    return hi, lo


def _install_wait_split(nc, limit=1):
    """This container's walrus encodes at most one sync-wait per
    instruction; hoist extra on_wait entries onto preceding NoOps."""
    orig = nc.to_json_bytes

    def fixed():
        m = json.loads(orig())
        n = 0
        for fn in m["functions"]:
            for bb in fn["blocks"]:
                out = []
                for inst in bb["instructions"]:
                    si = inst.get("sync_info") or {}
                    waits = si.get("on_wait") or []
                    while len(waits) > limit:
                        chunk, waits = waits[:limit], waits[limit:]
                        n += 1
                        out.append({
                            "debug": inst.get("debug"),
                            "engine": inst["engine"],
                            "ins": [], "outs": [],
                            "name": f"I-waitsplit-{n}",
                            "opcode": "NoOp",
                            "sync_info": {"on_update": [], "on_wait": chunk},
                        })
                    si["on_wait"] = waits
                    inst["sync_info"] = si
                    out.append(inst)
                bb["instructions"] = out
        return json.dumps(m).encode()

    nc.to_json_bytes = fixed


def _build_program(t):
    """Build the per-core Bass program. `t` is the softplus(log_t) power,
    baked in as an immediate."""
    t = float(t)
    nc = bass.Bass(num_devices=NCORES)

    rtab_in = nc.declare_dram_parameter("rtab", [KZ, M], BF16, isOutput=False)
    lx_in = nc.declare_dram_parameter("lx", [KFULL, SH], BF16, isOutput=False)
    lr_in = nc.declare_dram_parameter("lr", [KFULL, SH], BF16, isOutput=False)
    bx_in = nc.declare_dram_parameter("bx", [P, NST], F32, isOutput=False)
    br_in = nc.declare_dram_parameter("br", [P, NST], F32, isOutput=False)
    # bf16 output halves the PJRT zero-donation upload and the result
    # download over the axon tunnel; host upcasts to fp32.
    w_out = nc.declare_dram_parameter("out", [SH, M], BF16, isOutput=True)

    with TileContext(nc, num_cores=NCORES) as tc:
        with (
            tc.tile_pool(name="const", bufs=1) as const,
            tc.tile_pool(name="psum", bufs=2, space="PSUM") as psum,
            tc.tile_pool(name="epool", bufs=3) as epool,
            tc.tile_pool(name="tpool", bufs=1) as tpool,
            tc.tile_pool(name="opool", bufs=3) as opool,
            tc.tile_pool(name="dram", bufs=1, space="DRAM") as dram,
        ):
            rtab = const.tile([KZ, M], BF16)
            rtab2 = const.tile([2, M], BF16)        # lnDref hi/lo, device-filled
            ones2 = const.tile([2, P], BF16)        # K=2 all-ones stationary operand
            nc.gpsimd.memset(ones2[:], 1.0)
            lx = const.tile([KFULL, SH], BF16)
            lr = const.tile([KFULL, SH], BF16)
            bx = const.tile([P, NST], F32)
            br = const.tile([P, NST], F32)
            nc.sync.dma_start(out=rtab[:], in_=rtab_in[:])
            nc.sync.dma_start(out=lx[:], in_=lx_in[:])
            nc.sync.dma_start(out=lr[:], in_=lr_in[:])
            nc.sync.dma_start(out=bx[:], in_=bx_in[:])
            nc.sync.dma_start(out=br[:], in_=br_in[:])

            drefrep = const.tile([P, M], F32)
            dinvrep = const.tile([P, M], F32)

            # per-(stripe,block) activation accum columns
            sa = const.tile([P, NST * NCB], F32)
            sb = const.tile([P, NST * NCB], F32)
            sc1 = const.tile([P, NST * NCB], F32)
            sc2 = const.tile([P, NST * NCB], F32)
            # per-stripe stats
            lns1r = const.tile([P, NST], F32)
            dref_loc = const.tile([P, NST], F32)
            s2r = const.tile([P, NST], F32)
            lns2r = const.tile([P, NST], F32)
            qref = const.tile([P, NST], F32)
            dinv_loc = const.tile([P, NST], F32)
            s1r = const.tile([P, NST], F32)

            dref_dram = dram.tile([SH], F32)
            dref_g = dram.tile([M], F32)
            dinv_dram = dram.tile([SH], F32)
            dinv_g = dram.tile([M], F32)

            groups = [list(range(NCORES))]

            def zmm(zp, lhsT, st, blk, with_ln):
                """Fill psum tile zp[:, 0:CB] with z for stripe st, block blk."""
                for mm in range(CB // MMW):
                    col = blk * CB + mm * MMW
                    nc.tensor.matmul(
                        zp[:, mm * MMW:(mm + 1) * MMW],
                        lhsT[0:KZ, st * P:(st + 1) * P],
                        rtab[0:KZ, col:col + MMW],
                        start=True, stop=not with_ln,
                    )
                    if with_ln:
                        nc.tensor.matmul(
                            zp[:, mm * MMW:(mm + 1) * MMW],
                            ones2[:],
                            rtab2[0:2, col:col + MMW],
                            start=False, stop=True,
                        )

            # ---- phase A: ref rowsums -> Dref shard ----
            for st in range(NST):
                for blk in range(NCB):
                    zp = psum.tile([P, CB], F32, tag="zp")
                    zmm(zp, lr, st, blk, with_ln=False)
                    e = epool.tile([P, CB], F32, tag="e")
                    nc.scalar.activation(
                        e[:], zp[:], AF.Exp, bias=br[:, st:st + 1],
                        accum_out=sa[:, st * NCB + blk:st * NCB + blk + 1],
                    )
            nc.vector.tensor_reduce(
                s1r[:], sa[:].rearrange("p (s q) -> p s q", q=NCB),
                axis=mybir.AxisListType.X, op=OP.add,
            )
            nc.scalar.activation(lns1r[:], s1r[:], AF.Ln)
            nc.scalar.activation(dref_loc[:], lns1r[:], AF.Exp, scale=-t)
            # shard -> dram (global index j = core*SH + st*P + p)
            nc.sync.dma_start(
                out=dref_dram[:].rearrange("(s p) -> p s", p=P), in_=dref_loc[:]
            )
            nc.gpsimd.collective_compute(
                "AllGather", OP.bypass, replica_groups=groups,
                ins=[dref_dram[:]], outs=[dref_g[:]],
            )
            nc.sync.dma_start(out=drefrep[:], in_=dref_g[:].partition_broadcast(P))
            # lnDref hi/lo rows for the phase-B fold
            lnstage = const.tile([P, M // P], F32)
            lnfull = const.tile([P, M // P], F32)
            lnl = const.tile([P, M // P], F32)
            lnh_bf = const.tile([P, M // P], BF16)
            lnh_f = const.tile([P, M // P], F32)
            lnl_bf = const.tile([P, M // P], BF16)
            nc.sync.dma_start(
                out=lnstage[:], in_=dref_g[:].rearrange("(p c) -> p c", p=P)
            )
            nc.scalar.activation(lnfull[:], lnstage[:], AF.Ln)
            nc.vector.tensor_copy(lnh_bf[:], lnfull[:])
            nc.vector.tensor_copy(lnh_f[:], lnh_bf[:])
            nc.vector.tensor_tensor(
                out=lnl[:], in0=lnfull[:], in1=lnh_f[:], op=OP.subtract
            )
            nc.vector.tensor_copy(lnl_bf[:], lnl[:])
            nc.sync.dma_start(out=rtab2[0:1, :], in_=lnh_bf[:])
            nc.sync.dma_start(out=rtab2[1:2, :], in_=lnl_bf[:])

            # ---- phase B: Dref-weighted ref rowsums -> Dinv1ref shard ----
            for st in range(NST):
                for blk in range(NCB):
                    zp = psum.tile([P, CB], F32, tag="zp")
                    zmm(zp, lr, st, blk, with_ln=True)
                    e = epool.tile([P, CB], F32, tag="e")
                    nc.scalar.activation(
                        e[:], zp[:], AF.Exp, bias=br[:, st:st + 1],
                        accum_out=sb[:, st * NCB + blk:st * NCB + blk + 1],
                    )
            nc.vector.tensor_reduce(
                s2r[:], sb[:].rearrange("p (s q) -> p s q", q=NCB),
                axis=mybir.AxisListType.X, op=OP.add,
            )
            nc.scalar.activation(lns2r[:], s2r[:], AF.Ln)
            # Dinv1ref = exp(-0.5*(-t*lnS1r + lnS2r))
            nc.vector.scalar_tensor_tensor(
                out=qref[:], in0=lns1r[:], scalar=-t, in1=lns2r[:],
                op0=OP.mult, op1=OP.add,
            )
            nc.scalar.activation(dinv_loc[:], qref[:], AF.Exp, scale=-0.5)
            nc.sync.dma_start(
                out=dinv_dram[:].rearrange("(s p) -> p s", p=P), in_=dinv_loc[:]
            )
            nc.gpsimd.collective_compute(
                "AllGather", OP.bypass, replica_groups=groups,
                ins=[dinv_dram[:]], outs=[dinv_g[:]],
            )
            nc.sync.dma_start(out=dinvrep[:], in_=dinv_g[:].partition_broadcast(P))

            # ---- phase C: cross matrix, fused output ----
            for st in range(NST):
                tstripe = tpool.tile([P, M], F32, tag="t")
                for blk in range(NCB):
                    zp = psum.tile([P, CB], F32, tag="zp")
                    zmm(zp, lx, st, blk, with_ln=False)
                    e = epool.tile([P, CB], F32, tag="e")
                    nc.scalar.activation(
                        e[:], zp[:], AF.Exp, bias=bx[:, st:st + 1],
                        accum_out=sc1[:, st * NCB + blk:st * NCB + blk + 1],
                    )
                    # T = E * Dref_j ; S2 part = rowsum(T)
                    nc.vector.scalar_tensor_tensor(
                        out=tstripe[:, blk * CB:(blk + 1) * CB],
                        in0=e[:], scalar=1.0,
                        in1=drefrep[:, blk * CB:(blk + 1) * CB],
                        op0=OP.mult, op1=OP.mult,
                        accum_out=sc2[:, st * NCB + blk:st * NCB + blk + 1],
                    )
                s1 = const.tile([P, 1], F32, tag=f"s1_{st}")
                s2 = const.tile([P, 1], F32, tag=f"s2_{st}")
                l1 = const.tile([P, 1], F32, tag=f"l1_{st}")
                l2 = const.tile([P, 1], F32, tag=f"l2_{st}")
                q = const.tile([P, 1], F32, tag=f"q_{st}")
                r = const.tile([P, 1], F32, tag=f"r_{st}")
                nc.vector.tensor_reduce(
                    s1[:], sc1[:, st * NCB:(st + 1) * NCB],
                    axis=mybir.AxisListType.X, op=OP.add,
                )
                nc.vector.tensor_reduce(
                    s2[:], sc2[:, st * NCB:(st + 1) * NCB],
                    axis=mybir.AxisListType.X, op=OP.add,
                )
                nc.scalar.activation(l1[:], s1[:], AF.Ln)
                nc.scalar.activation(l2[:], s2[:], AF.Ln)
                # r = exp(-0.5*(t*lnS1 + lnS2))
                nc.vector.scalar_tensor_tensor(
                    out=q[:], in0=l1[:], scalar=t, in1=l2[:],
                    op0=OP.mult, op1=OP.add,
                )
                nc.scalar.activation(r[:], q[:], AF.Exp, scale=-0.5)
                for blk in range(NCB):
                    o = opool.tile([P, CB], BF16, tag="o")
                    nc.vector.scalar_tensor_tensor(
                        out=o[:], in0=tstripe[:, blk * CB:(blk + 1) * CB],
                        scalar=r[:], in1=dinvrep[:, blk * CB:(blk + 1) * CB],
                        op0=OP.mult, op1=OP.mult,
                    )
                    nc.sync.dma_start(
                        out=w_out[st * P:(st + 1) * P, blk * CB:(blk + 1) * CB],
                        in_=o[:],
                    )

    _install_wait_split(nc)
    return nc


def _prep_inputs(X, Xref, s):
    """Host-side O((N+M)*D) prep of the augmented bf16 operand tables."""
    X = np.asarray(X, dtype=np.float32)
    Xref = np.asarray(Xref, dtype=np.float32)
    s = np.float32(s)

    # moving-side table: b = 2s * xref, plus -s*||xref||^2 rows
    b = (2.0 * s) * Xref.T                      # (16, M)
    bh, bl = _hilo(b)
    bn = -(s * np.sum(Xref * Xref, axis=1))     # (M,)
    bnh, bnl = _hilo(bn)
    rtab = np.zeros((KZ, M), dtype=ml_dtypes.bfloat16)
    rtab[0:D] = bh
    rtab[D:2 * D] = bl
    rtab[2 * D:3 * D] = bh
    rtab[KXY] = bnh
    rtab[KXY + 1] = bnl

    def lhs_table(A):
        a = A.T                                  # (16, rows)
        ah, al = _hilo(a)
        tab = np.ones((KFULL, A.shape[0]), dtype=ml_dtypes.bfloat16)
        tab[0:D] = ah
        tab[D:2 * D] = ah
        tab[2 * D:3 * D] = al
        return tab

    def bias_table(A):
        v = -(s * np.sum(A * A, axis=1))         # (rows,)
        return np.ascontiguousarray(v.reshape(NST, P).T)   # (P, NST)

    return rtab, lhs_table, bias_table


def _install_fast_fetch():
    """Fetch sharded jax Arrays shard-by-shard instead of through
    ArrayImpl._value's global-assembly path.

    On this single-core host the stock sharded `_value` measures
    ~2.9s for the kernel's 134MB output vs ~2.2s for issuing
    copy_to_host_async on every shard and assembling from per-shard
    `__array__` (3/3 trials each). The replacement below produces a
    byte-identical, read-only, cached array exactly like the original
    and falls back to the stock path for replicated / non-addressable
    arrays or on any error."""
    from jax._src import array as jarray

    ArrayImpl = jarray.ArrayImpl
    orig = ArrayImpl._value

    def _value_fast(self):
        if self._npy_value is None:
            try:
                if (not self.is_fully_replicated) and self.is_fully_addressable:
                    shards = self.addressable_shards
                    for sh in shards:
                        sh.data.copy_to_host_async()
                    out = np.empty(self.shape, self.dtype)
                    for sh in shards:
                        out[sh.index] = np.asarray(sh.data)
                    out.flags.writeable = False
                    self._npy_value = out
                    return out
            except Exception:
                pass
        return orig.fget(self)

    ArrayImpl._value = property(_value_fast)


_install_fast_fetch()

_prog_cache = {}


def kernel(X, Xref, log_eps, log_t):
    X = np.asarray(X, dtype=np.float32)
    Xref = np.asarray(Xref, dtype=np.float32)
    eps = _softplus(np.float32(log_eps))
    t = _softplus(np.float32(log_t))
    s = np.float32(1.0 / (4.0 * eps))

    key = (float(t),)
    if key not in _prog_cache:
        _prog_cache[key] = _build_program(t)
    nc = _prog_cache[key]

    rtab, lhs_table, bias_table = _prep_inputs(X, Xref, s)

    in_maps = []
    for k in range(NCORES):
        xs = X[k * SH:(k + 1) * SH]
        rs = Xref[k * SH:(k + 1) * SH]
        in_maps.append({
            "rtab": rtab,
            "lx": lhs_table(xs),
            "lr": lhs_table(rs),
            "bx": bias_table(xs),
            "br": bias_table(rs),
        })

    res = run_bass_kernel_spmd(nc, in_maps, list(range(NCORES)))
    global _last_results
    _last_results = res
    out = np.empty((N, M), dtype=np.float32)
    for k in range(NCORES):
        out[k * SH:(k + 1) * SH] = res.results[k]["out"]
    return out


_last_results = None



# revision 9
# speedup vs baseline: 1.0158x; 1.0158x over previous
"""Trainium2 Bass kernel for the KernelScDM problem (8-core SPMD).

Computes, for X (N,16) and Xref (M,16) with N=M=8192:
  W0    = exp(-||x_i - xref_j||^2 / (4 eps))          (N,M)
  Dref  = rowsum(rbf(Xref,Xref))^-t                   (M,)
  Dinv1ref = (Dref * (Wr@Dref))^-0.5                  (M,)
  Dx    = rowsum(W0)^-t ; Dinv1x = (Dx * (W0@Dref))^-0.5
  W     = Dinv1x[:,None]*Dx[:,None] * W0 * Dref[None,:]*Dinv1ref[None,:]

Sharding: rows of X (and of the Xref x Xref reference matrix) split
across 8 cores; Dref / Dinv1ref shards exchanged with two AllGathers.

The -s*d2 kernel argument is produced on the PE as one matmul over
augmented inputs, with fp32 accuracy recovered from bf16 operands via a
hi/lo split (a.b ~= ah.bh + ah.bl + al.bh). exp runs on ACT with fused
row-sum accumulation; the Dref-weighted row-sum and the final scaling
run as single fused scalar_tensor_tensor ops on the DVE.

Wall-clock here is dominated by the axon tunnel (~25-45 MB/s), not the
device: the result matrix crosses the wire once down (plus a same-size
zero-donation staging cost up, imposed by run_bass_via_pjrt). The
device therefore emits W in bf16 — halving both leg costs vs fp32 —
and the host upcasts to fp32 during the unshard. bf16 keeps elementwise
error ~4e-3, comfortably inside the 2e-2 gate; fp8 (~6% elementwise)
would not pass, and sub-16-bit packing loses its wire savings to
host-side decode.
"""

import json

import numpy as np
import ml_dtypes

import concourse.bass as bass
import concourse.mybir as mybir
from concourse.tile import TileContext
from concourse.bass_utils import run_bass_kernel_spmd

F32 = mybir.dt.float32
BF16 = mybir.dt.bfloat16
AF = mybir.ActivationFunctionType
OP = mybir.AluOpType

N = 8192
M = 8192
D = 16
NCORES = 8
SH = N // NCORES          # rows per core
P = 128                   # partitions
NST = SH // P             # stripes per core (8)
CB = 2048                 # column block (psum tile width)
NCB = M // CB             # column blocks (4)
MMW = 512                 # single-matmul moving width
KXY = 3 * D               # hi/lo split-K rows for the dot product (48)
KZ = KXY + 2              # + norm-term hi/lo rows (50)
KFULL = KZ + 2            # + lnDref hi/lo rows, phase B only (52)


def _softplus(x):
    x = np.float32(x)
    return np.float32(np.log1p(np.exp(-abs(x))) + max(x, 0.0))


def _hilo(v):
    """Split fp32 array into (hi, lo) bf16 parts; hi+lo ~ v to ~16 mantissa bits."""
    hi = v.astype(ml_dtypes.bfloat16)
    lo = (v - hi.astype(np.float32)).astype(ml_dtypes.bfloat16)
    return hi, lo


def _install_wait_split(nc, limit=1):
    """This container's walrus encodes at most one sync-wait per
    instruction; hoist extra on_wait entries onto preceding NoOps."""
    orig = nc.to_json_bytes

    def fixed():
        m = json.loads(orig())
        n = 0
        for fn in m["functions"]:
            for bb in fn["blocks"]:
                out = []
                for inst in bb["instructions"]:
                    si = inst.get("sync_info") or {}
                    waits = si.get("on_wait") or []
                    while len(waits) > limit:
                        chunk, waits = waits[:limit], waits[limit:]
                        n += 1
                        out.append({
                            "debug": inst.get("debug"),
                            "engine": inst["engine"],
                            "ins": [], "outs": [],
                            "name": f"I-waitsplit-{n}",
                            "opcode": "NoOp",
                            "sync_info": {"on_update": [], "on_wait": chunk},
                        })
                    si["on_wait"] = waits
                    inst["sync_info"] = si
                    out.append(inst)
                bb["instructions"] = out
        return json.dumps(m).encode()

    nc.to_json_bytes = fixed


def _build_program(t):
    """Build the per-core Bass program. `t` is the softplus(log_t) power,
    baked in as an immediate."""
    t = float(t)
    nc = bass.Bass(num_devices=NCORES)

    rtab_in = nc.declare_dram_parameter("rtab", [KZ, M], BF16, isOutput=False)
    lx_in = nc.declare_dram_parameter("lx", [KFULL, SH], BF16, isOutput=False)
    lr_in = nc.declare_dram_parameter("lr", [KFULL, SH], BF16, isOutput=False)
    bx_in = nc.declare_dram_parameter("bx", [P, NST], F32, isOutput=False)
    br_in = nc.declare_dram_parameter("br", [P, NST], F32, isOutput=False)
    # bf16 output halves the PJRT zero-donation upload and the result
    # download over the axon tunnel; host upcasts to fp32.
    w_out = nc.declare_dram_parameter("out", [SH, M], BF16, isOutput=True)

    with TileContext(nc, num_cores=NCORES) as tc:
        with (
            tc.tile_pool(name="const", bufs=1) as const,
            tc.tile_pool(name="psum", bufs=2, space="PSUM") as psum,
            tc.tile_pool(name="epool", bufs=3) as epool,
            tc.tile_pool(name="tpool", bufs=1) as tpool,
            tc.tile_pool(name="opool", bufs=3) as opool,
            tc.tile_pool(name="dram", bufs=1, space="DRAM") as dram,
        ):
            rtab = const.tile([KZ, M], BF16)
            rtab2 = const.tile([2, M], BF16)        # lnDref hi/lo, device-filled
            ones2 = const.tile([2, P], BF16)        # K=2 all-ones stationary operand
            nc.gpsimd.memset(ones2[:], 1.0)
            lx = const.tile([KFULL, SH], BF16)
            lr = const.tile([KFULL, SH], BF16)
            bx = const.tile([P, NST], F32)
            br = const.tile([P, NST], F32)
            nc.sync.dma_start(out=rtab[:], in_=rtab_in[:])
            nc.sync.dma_start(out=lx[:], in_=lx_in[:])
            nc.sync.dma_start(out=lr[:], in_=lr_in[:])
            nc.sync.dma_start(out=bx[:], in_=bx_in[:])
            nc.sync.dma_start(out=br[:], in_=br_in[:])

            drefrep = const.tile([P, M], F32)
            dinvrep = const.tile([P, M], F32)

            # per-(stripe,block) activation accum columns
            sa = const.tile([P, NST * NCB], F32)
            sb = const.tile([P, NST * NCB], F32)
            sc1 = const.tile([P, NST * NCB], F32)
            sc2 = const.tile([P, NST * NCB], F32)
            # per-stripe stats
            lns1r = const.tile([P, NST], F32)
            dref_loc = const.tile([P, NST], F32)
            s2r = const.tile([P, NST], F32)
            lns2r = const.tile([P, NST], F32)
            qref = const.tile([P, NST], F32)
            dinv_loc = const.tile([P, NST], F32)
            s1r = const.tile([P, NST], F32)

            dref_dram = dram.tile([SH], F32)
            dref_g = dram.tile([M], F32)
            dinv_dram = dram.tile([SH], F32)
            dinv_g = dram.tile([M], F32)

            groups = [list(range(NCORES))]

            def zmm(zp, lhsT, st, blk, with_ln):
                """Fill psum tile zp[:, 0:CB] with z for stripe st, block blk."""
                for mm in range(CB // MMW):
                    col = blk * CB + mm * MMW
                    nc.tensor.matmul(
                        zp[:, mm * MMW:(mm + 1) * MMW],
                        lhsT[0:KZ, st * P:(st + 1) * P],
                        rtab[0:KZ, col:col + MMW],
                        start=True, stop=not with_ln,
                    )
                    if with_ln:
                        nc.tensor.matmul(
                            zp[:, mm * MMW:(mm + 1) * MMW],
                            ones2[:],
                            rtab2[0:2, col:col + MMW],
                            start=False, stop=True,
                        )

            # ---- phase A: ref rowsums -> Dref shard ----
            for st in range(NST):
                for blk in range(NCB):
                    zp = psum.tile([P, CB], F32, tag="zp")
                    zmm(zp, lr, st, blk, with_ln=False)
                    e = epool.tile([P, CB], F32, tag="e")
                    nc.scalar.activation(
                        e[:], zp[:], AF.Exp, bias=br[:, st:st + 1],
                        accum_out=sa[:, st * NCB + blk:st * NCB + blk + 1],
                    )
            nc.vector.tensor_reduce(
                s1r[:], sa[:].rearrange("p (s q) -> p s q", q=NCB),
                axis=mybir.AxisListType.X, op=OP.add,
            )
            nc.scalar.activation(lns1r[:], s1r[:], AF.Ln)
            nc.scalar.activation(dref_loc[:], lns1r[:], AF.Exp, scale=-t)
            # shard -> dram (global index j = core*SH + st*P + p)
            nc.sync.dma_start(
                out=dref_dram[:].rearrange("(s p) -> p s", p=P), in_=dref_loc[:]
            )
            nc.gpsimd.collective_compute(
                "AllGather", OP.bypass, replica_groups=groups,
                ins=[dref_dram[:]], outs=[dref_g[:]],
            )
            nc.sync.dma_start(out=drefrep[:], in_=dref_g[:].partition_broadcast(P))
            # lnDref hi/lo rows for the phase-B fold
            lnstage = const.tile([P, M // P], F32)
            lnfull = const.tile([P, M // P], F32)
            lnl = const.tile([P, M // P], F32)
            lnh_bf = const.tile([P, M // P], BF16)
            lnh_f = const.tile([P, M // P], F32)
            lnl_bf = const.tile([P, M // P], BF16)
            nc.sync.dma_start(
                out=lnstage[:], in_=dref_g[:].rearrange("(p c) -> p c", p=P)
            )
            nc.scalar.activation(lnfull[:], lnstage[:], AF.Ln)
            nc.vector.tensor_copy(lnh_bf[:], lnfull[:])
            nc.vector.tensor_copy(lnh_f[:], lnh_bf[:])
            nc.vector.tensor_tensor(
                out=lnl[:], in0=lnfull[:], in1=lnh_f[:], op=OP.subtract
            )
            nc.vector.tensor_copy(lnl_bf[:], lnl[:])
            nc.sync.dma_start(out=rtab2[0:1, :], in_=lnh_bf[:])
            nc.sync.dma_start(out=rtab2[1:2, :], in_=lnl_bf[:])

            # ---- phase B: Dref-weighted ref rowsums -> Dinv1ref shard ----
            for st in range(NST):
                for blk in range(NCB):
                    zp = psum.tile([P, CB], F32, tag="zp")
                    zmm(zp, lr, st, blk, with_ln=True)
                    e = epool.tile([P, CB], F32, tag="e")
                    nc.scalar.activation(
                        e[:], zp[:], AF.Exp, bias=br[:, st:st + 1],
                        accum_out=sb[:, st * NCB + blk:st * NCB + blk + 1],
                    )
            nc.vector.tensor_reduce(
                s2r[:], sb[:].rearrange("p (s q) -> p s q", q=NCB),
                axis=mybir.AxisListType.X, op=OP.add,
            )
            nc.scalar.activation(lns2r[:], s2r[:], AF.Ln)
            # Dinv1ref = exp(-0.5*(-t*lnS1r + lnS2r))
            nc.vector.scalar_tensor_tensor(
                out=qref[:], in0=lns1r[:], scalar=-t, in1=lns2r[:],
                op0=OP.mult, op1=OP.add,
            )
            nc.scalar.activation(dinv_loc[:], qref[:], AF.Exp, scale=-0.5)
            nc.sync.dma_start(
                out=dinv_dram[:].rearrange("(s p) -> p s", p=P), in_=dinv_loc[:]
            )
            nc.gpsimd.collective_compute(
                "AllGather", OP.bypass, replica_groups=groups,
                ins=[dinv_dram[:]], outs=[dinv_g[:]],
            )
            nc.sync.dma_start(out=dinvrep[:], in_=dinv_g[:].partition_broadcast(P))

            # ---- phase C: cross matrix, fused output ----
            for st in range(NST):
                tstripe = tpool.tile([P, M], F32, tag="t")
                for blk in range(NCB):
                    zp = psum.tile([P, CB], F32, tag="zp")
                    zmm(zp, lx, st, blk, with_ln=False)
                    e = epool.tile([P, CB], F32, tag="e")
                    nc.scalar.activation(
                        e[:], zp[:], AF.Exp, bias=bx[:, st:st + 1],
                        accum_out=sc1[:, st * NCB + blk:st * NCB + blk + 1],
                    )
                    # T = E * Dref_j ; S2 part = rowsum(T)
                    nc.vector.scalar_tensor_tensor(
                        out=tstripe[:, blk * CB:(blk + 1) * CB],
                        in0=e[:], scalar=1.0,
                        in1=drefrep[:, blk * CB:(blk + 1) * CB],
                        op0=OP.mult, op1=OP.mult,
                        accum_out=sc2[:, st * NCB + blk:st * NCB + blk + 1],
                    )
                s1 = const.tile([P, 1], F32, tag=f"s1_{st}")
                s2 = const.tile([P, 1], F32, tag=f"s2_{st}")
                l1 = const.tile([P, 1], F32, tag=f"l1_{st}")
                l2 = const.tile([P, 1], F32, tag=f"l2_{st}")
                q = const.tile([P, 1], F32, tag=f"q_{st}")
                r = const.tile([P, 1], F32, tag=f"r_{st}")
                nc.vector.tensor_reduce(
                    s1[:], sc1[:, st * NCB:(st + 1) * NCB],
                    axis=mybir.AxisListType.X, op=OP.add,
                )
                nc.vector.tensor_reduce(
                    s2[:], sc2[:, st * NCB:(st + 1) * NCB],
                    axis=mybir.AxisListType.X, op=OP.add,
                )
                nc.scalar.activation(l1[:], s1[:], AF.Ln)
                nc.scalar.activation(l2[:], s2[:], AF.Ln)
                # r = exp(-0.5*(t*lnS1 + lnS2))
                nc.vector.scalar_tensor_tensor(
                    out=q[:], in0=l1[:], scalar=t, in1=l2[:],
                    op0=OP.mult, op1=OP.add,
                )
                nc.scalar.activation(r[:], q[:], AF.Exp, scale=-0.5)
                for blk in range(NCB):
                    o = opool.tile([P, CB], BF16, tag="o")
                    nc.vector.scalar_tensor_tensor(
                        out=o[:], in0=tstripe[:, blk * CB:(blk + 1) * CB],
                        scalar=r[:], in1=dinvrep[:, blk * CB:(blk + 1) * CB],
                        op0=OP.mult, op1=OP.mult,
                    )
                    nc.sync.dma_start(
                        out=w_out[st * P:(st + 1) * P, blk * CB:(blk + 1) * CB],
                        in_=o[:],
                    )

    _install_wait_split(nc)
    return nc


def _prep_inputs(X, Xref, s):
    """Host-side O((N+M)*D) prep of the augmented bf16 operand tables."""
    X = np.asarray(X, dtype=np.float32)
    Xref = np.asarray(Xref, dtype=np.float32)
    s = np.float32(s)

    # moving-side table: b = 2s * xref, plus -s*||xref||^2 rows
    b = (2.0 * s) * Xref.T                      # (16, M)
    bh, bl = _hilo(b)
    bn = -(s * np.sum(Xref * Xref, axis=1))     # (M,)
    bnh, bnl = _hilo(bn)
    rtab = np.zeros((KZ, M), dtype=ml_dtypes.bfloat16)
    rtab[0:D] = bh
    rtab[D:2 * D] = bl
    rtab[2 * D:3 * D] = bh
    rtab[KXY] = bnh
    rtab[KXY + 1] = bnl

    def lhs_table(A):
        a = A.T                                  # (16, rows)
        ah, al = _hilo(a)
        tab = np.ones((KFULL, A.shape[0]), dtype=ml_dtypes.bfloat16)
        tab[0:D] = ah
        tab[D:2 * D] = ah
        tab[2 * D:3 * D] = al
        return tab

    def bias_table(A):
        v = -(s * np.sum(A * A, axis=1))         # (rows,)
        return np.ascontiguousarray(v.reshape(NST, P).T)   # (P, NST)

    return rtab, lhs_table, bias_table


_prog_cache = {}


def kernel(X, Xref, log_eps, log_t):
    X = np.asarray(X, dtype=np.float32)
    Xref = np.asarray(Xref, dtype=np.float32)
    eps = _softplus(np.float32(log_eps))
    t = _softplus(np.float32(log_t))
    s = np.float32(1.0 / (4.0 * eps))

    key = (float(t),)
    if key not in _prog_cache:
        _prog_cache[key] = _build_program(t)
    nc = _prog_cache[key]

    rtab, lhs_table, bias_table = _prep_inputs(X, Xref, s)

    in_maps = []
    for k in range(NCORES):
        xs = X[k * SH:(k + 1) * SH]
        rs = Xref[k * SH:(k + 1) * SH]
        in_maps.append({
            "rtab": rtab,
            "lx": lhs_table(xs),
            "lr": lhs_table(rs),
            "bx": bias_table(xs),
            "br": bias_table(rs),
        })

    res = run_bass_kernel_spmd(nc, in_maps, list(range(NCORES)))
    global _last_results
    _last_results = res
    out = np.empty((N, M), dtype=np.float32)
    for k in range(NCORES):
        out[k * SH:(k + 1) * SH] = res.results[k]["out"]
    return out


_last_results = None



# revision 15
# speedup vs baseline: 1.0480x; 1.0318x over previous
"""Trainium2 Bass kernel for the KernelScDM problem (8-core SPMD).

Computes, for X (N,16) and Xref (M,16) with N=M=8192:
  W0    = exp(-||x_i - xref_j||^2 / (4 eps))          (N,M)
  Dref  = rowsum(rbf(Xref,Xref))^-t                   (M,)
  Dinv1ref = (Dref * (Wr@Dref))^-0.5                  (M,)
  Dx    = rowsum(W0)^-t ; Dinv1x = (Dx * (W0@Dref))^-0.5
  W     = Dinv1x[:,None]*Dx[:,None] * W0 * Dref[None,:]*Dinv1ref[None,:]

Sharding: rows of X (and of the Xref x Xref reference matrix) split
across 8 cores; Dref / Dinv1ref shards exchanged with two AllGathers.

The -s*d2 kernel argument is produced on the PE as one matmul over
augmented inputs, with fp32 accuracy recovered from bf16 operands via a
hi/lo split (a.b ~= ah.bh + ah.bl + al.bh). exp runs on ACT with fused
row-sum accumulation; the Dref-weighted row-sum and the final scaling
run as single fused scalar_tensor_tensor ops on the DVE.

Wall-clock here is dominated by the axon tunnel (~25-45 MB/s), not the
device: the result matrix crosses the wire once down (plus a same-size
zero-donation staging cost up, imposed by run_bass_via_pjrt). The
device therefore emits W in bf16 — halving both leg costs vs fp32 —
and the host upcasts to fp32 during the unshard. bf16 keeps elementwise
error ~4e-3, comfortably inside the 2e-2 gate; fp8 (~6% elementwise)
would not pass, and sub-16-bit packing loses its wire savings to
host-side decode.
"""

import json

import numpy as np
import ml_dtypes

import concourse.bass as bass
import concourse.mybir as mybir
from concourse.tile import TileContext
from concourse.bass_utils import run_bass_kernel_spmd

F32 = mybir.dt.float32
BF16 = mybir.dt.bfloat16
AF = mybir.ActivationFunctionType
OP = mybir.AluOpType

N = 8192
M = 8192
D = 16
NCORES = 8
SH = N // NCORES          # rows per core
P = 128                   # partitions
NST = SH // P             # stripes per core (8)
CB = 2048                 # column block (psum tile width)
NCB = M // CB             # column blocks (4)
MMW = 512                 # single-matmul moving width
KXY = 3 * D               # hi/lo split-K rows for the dot product (48)
KZ = KXY + 2              # + norm-term hi/lo rows (50)
KFULL = KZ + 2            # + lnDref hi/lo rows, phase B only (52)


def _softplus(x):
    x = np.float32(x)
    return np.float32(np.log1p(np.exp(-abs(x))) + max(x, 0.0))


def _hilo(v):
    """Split fp32 array into (hi, lo) bf16 parts; hi+lo ~ v to ~16 mantissa bits."""
    hi = v.astype(ml_dtypes.bfloat16)
    lo = (v - hi.astype(np.float32)).astype(ml_dtypes.bfloat16)
    return hi, lo


def _install_wait_split(nc, limit=1):
    """This container's walrus encodes at most one sync-wait per
    instruction; hoist extra on_wait entries onto preceding NoOps."""
    orig = nc.to_json_bytes

    def fixed():
        m = json.loads(orig())
        n = 0
        for fn in m["functions"]:
            for bb in fn["blocks"]:
                out = []
                for inst in bb["instructions"]:
                    si = inst.get("sync_info") or {}
                    waits = si.get("on_wait") or []
                    while len(waits) > limit:
                        chunk, waits = waits[:limit], waits[limit:]
                        n += 1
                        out.append({
                            "debug": inst.get("debug"),
                            "engine": inst["engine"],
                            "ins": [], "outs": [],
                            "name": f"I-waitsplit-{n}",
                            "opcode": "NoOp",
                            "sync_info": {"on_update": [], "on_wait": chunk},
                        })
                    si["on_wait"] = waits
                    inst["sync_info"] = si
                    out.append(inst)
                bb["instructions"] = out
        return json.dumps(m).encode()

    nc.to_json_bytes = fixed


def _build_program(t):
    """Build the per-core Bass program. `t` is the softplus(log_t) power,
    baked in as an immediate."""
    t = float(t)
    nc = bass.Bass(num_devices=NCORES)

    # deduped uploads: rtab ships [bh(16), bl(16), bn(2)] and the lhs
    # tables ship [ah(16), al(16)]; the duplicated rows of the K=50
    # matmul layout (bh again, ah again) are filled by extra SBUF DMAs.
    rtab_in = nc.declare_dram_parameter("rtab", [2 * D + 2, M], BF16,
                                        isOutput=False)
    lx_in = nc.declare_dram_parameter("lx", [2 * D + 2, SH], BF16,
                                      isOutput=False)
    lr_in = nc.declare_dram_parameter("lr", [2 * D + 2, SH], BF16,
                                      isOutput=False)
    bx_in = nc.declare_dram_parameter("bx", [P, NST], F32, isOutput=False)
    br_in = nc.declare_dram_parameter("br", [P, NST], F32, isOutput=False)
    # bf16 output halves the PJRT zero-donation upload and the result
    # download over the axon tunnel; host upcasts to fp32.
    w_out = nc.declare_dram_parameter("out", [SH, M], BF16, isOutput=True)

    with TileContext(nc, num_cores=NCORES) as tc:
        with (
            tc.tile_pool(name="const", bufs=1) as const,
            tc.tile_pool(name="psum", bufs=2, space="PSUM") as psum,
            tc.tile_pool(name="epool", bufs=3) as epool,
            tc.tile_pool(name="tpool", bufs=1) as tpool,
            tc.tile_pool(name="opool", bufs=3) as opool,
            tc.tile_pool(name="dram", bufs=1, space="DRAM") as dram,
        ):
            rtab = const.tile([KZ, M], BF16)
            rtab2 = const.tile([2, M], BF16)        # lnDref hi/lo, device-filled
            ones2 = const.tile([2, P], BF16)        # K=2 all-ones stationary operand
            nc.gpsimd.memset(ones2[:], 1.0)
            lx = const.tile([KZ, SH], BF16)
            lr = const.tile([KZ, SH], BF16)
            bx = const.tile([P, NST], F32)
            br = const.tile([P, NST], F32)
            # expand deduped uploads into the K=50 matmul layout:
            # rtab rows [bh, bl, bh, bn2], lhs rows [ah, ah, al, ones2]
            nc.sync.dma_start(out=rtab[0:2 * D, :], in_=rtab_in[0:2 * D, :])
            nc.sync.dma_start(out=rtab[2 * D:3 * D, :], in_=rtab_in[0:D, :])
            nc.sync.dma_start(out=rtab[KXY:KZ, :], in_=rtab_in[2 * D:2 * D + 2, :])
            for tile, src in ((lx, lx_in), (lr, lr_in)):
                nc.sync.dma_start(out=tile[0:D, :], in_=src[0:D, :])
                nc.sync.dma_start(out=tile[D:2 * D, :], in_=src[0:D, :])
                nc.sync.dma_start(out=tile[2 * D:3 * D, :], in_=src[D:2 * D, :])
                nc.sync.dma_start(out=tile[KXY:KZ, :], in_=src[2 * D:2 * D + 2, :])
            nc.sync.dma_start(out=bx[:], in_=bx_in[:])
            nc.sync.dma_start(out=br[:], in_=br_in[:])

            drefrep = const.tile([P, M], F32)
            dinvrep = const.tile([P, M], F32)

            # per-(stripe,block) activation accum columns
            sa = const.tile([P, NST * NCB], F32)
            sb = const.tile([P, NST * NCB], F32)
            sc1 = const.tile([P, NST * NCB], F32)
            sc2 = const.tile([P, NST * NCB], F32)
            # per-stripe stats
            lns1r = const.tile([P, NST], F32)
            dref_loc = const.tile([P, NST], F32)
            s2r = const.tile([P, NST], F32)
            lns2r = const.tile([P, NST], F32)
            qref = const.tile([P, NST], F32)
            dinv_loc = const.tile([P, NST], F32)
            s1r = const.tile([P, NST], F32)

            dref_dram = dram.tile([SH], F32)
            dref_g = dram.tile([M], F32)
            dinv_dram = dram.tile([SH], F32)
            dinv_g = dram.tile([M], F32)

            groups = [list(range(NCORES))]

            def zmm(zp, lhsT, st, blk, with_ln):
                """Fill psum tile zp[:, 0:CB] with z for stripe st, block blk."""
                for mm in range(CB // MMW):
                    col = blk * CB + mm * MMW
                    nc.tensor.matmul(
                        zp[:, mm * MMW:(mm + 1) * MMW],
                        lhsT[0:KZ, st * P:(st + 1) * P],
                        rtab[0:KZ, col:col + MMW],
                        start=True, stop=not with_ln,
                    )
                    if with_ln:
                        nc.tensor.matmul(
                            zp[:, mm * MMW:(mm + 1) * MMW],
                            ones2[:],
                            rtab2[0:2, col:col + MMW],
                            start=False, stop=True,
                        )

            # ---- phase A: ref rowsums -> Dref shard ----
            for st in range(NST):
                for blk in range(NCB):
                    zp = psum.tile([P, CB], F32, tag="zp")
                    zmm(zp, lr, st, blk, with_ln=False)
                    e = epool.tile([P, CB], F32, tag="e")
                    nc.scalar.activation(
                        e[:], zp[:], AF.Exp, bias=br[:, st:st + 1],
                        accum_out=sa[:, st * NCB + blk:st * NCB + blk + 1],
                    )
            nc.vector.tensor_reduce(
                s1r[:], sa[:].rearrange("p (s q) -> p s q", q=NCB),
                axis=mybir.AxisListType.X, op=OP.add,
            )
            nc.scalar.activation(lns1r[:], s1r[:], AF.Ln)
            nc.scalar.activation(dref_loc[:], lns1r[:], AF.Exp, scale=-t)
            # shard -> dram (global index j = core*SH + st*P + p)
            nc.sync.dma_start(
                out=dref_dram[:].rearrange("(s p) -> p s", p=P), in_=dref_loc[:]
            )
            nc.gpsimd.collective_compute(
                "AllGather", OP.bypass, replica_groups=groups,
                ins=[dref_dram[:]], outs=[dref_g[:]],
            )
            nc.sync.dma_start(out=drefrep[:], in_=dref_g[:].partition_broadcast(P))
            # lnDref hi/lo rows for the phase-B fold
            lnstage = const.tile([P, M // P], F32)
            lnfull = const.tile([P, M // P], F32)
            lnl = const.tile([P, M // P], F32)
            lnh_bf = const.tile([P, M // P], BF16)
            lnh_f = const.tile([P, M // P], F32)
            lnl_bf = const.tile([P, M // P], BF16)
            nc.sync.dma_start(
                out=lnstage[:], in_=dref_g[:].rearrange("(p c) -> p c", p=P)
            )
            nc.scalar.activation(lnfull[:], lnstage[:], AF.Ln)
            nc.vector.tensor_copy(lnh_bf[:], lnfull[:])
            nc.vector.tensor_copy(lnh_f[:], lnh_bf[:])
            nc.vector.tensor_tensor(
                out=lnl[:], in0=lnfull[:], in1=lnh_f[:], op=OP.subtract
            )
            nc.vector.tensor_copy(lnl_bf[:], lnl[:])
            nc.sync.dma_start(out=rtab2[0:1, :], in_=lnh_bf[:])
            nc.sync.dma_start(out=rtab2[1:2, :], in_=lnl_bf[:])

            # ---- phase B: Dref-weighted ref rowsums -> Dinv1ref shard ----
            for st in range(NST):
                for blk in range(NCB):
                    zp = psum.tile([P, CB], F32, tag="zp")
                    zmm(zp, lr, st, blk, with_ln=True)
                    e = epool.tile([P, CB], F32, tag="e")
                    nc.scalar.activation(
                        e[:], zp[:], AF.Exp, bias=br[:, st:st + 1],
                        accum_out=sb[:, st * NCB + blk:st * NCB + blk + 1],
                    )
            nc.vector.tensor_reduce(
                s2r[:], sb[:].rearrange("p (s q) -> p s q", q=NCB),
                axis=mybir.AxisListType.X, op=OP.add,
            )
            nc.scalar.activation(lns2r[:], s2r[:], AF.Ln)
            # Dinv1ref = exp(-0.5*(-t*lnS1r + lnS2r))
            nc.vector.scalar_tensor_tensor(
                out=qref[:], in0=lns1r[:], scalar=-t, in1=lns2r[:],
                op0=OP.mult, op1=OP.add,
            )
            nc.scalar.activation(dinv_loc[:], qref[:], AF.Exp, scale=-0.5)
            nc.sync.dma_start(
                out=dinv_dram[:].rearrange("(s p) -> p s", p=P), in_=dinv_loc[:]
            )
            nc.gpsimd.collective_compute(
                "AllGather", OP.bypass, replica_groups=groups,
                ins=[dinv_dram[:]], outs=[dinv_g[:]],
            )
            nc.sync.dma_start(out=dinvrep[:], in_=dinv_g[:].partition_broadcast(P))

            # ---- phase C: cross matrix, fused output ----
            for st in range(NST):
                tstripe = tpool.tile([P, M], F32, tag="t")
                for blk in range(NCB):
                    zp = psum.tile([P, CB], F32, tag="zp")
                    zmm(zp, lx, st, blk, with_ln=False)
                    e = epool.tile([P, CB], F32, tag="e")
                    nc.scalar.activation(
                        e[:], zp[:], AF.Exp, bias=bx[:, st:st + 1],
                        accum_out=sc1[:, st * NCB + blk:st * NCB + blk + 1],
                    )
                    # T = E * Dref_j ; S2 part = rowsum(T)
                    nc.vector.scalar_tensor_tensor(
                        out=tstripe[:, blk * CB:(blk + 1) * CB],
                        in0=e[:], scalar=1.0,
                        in1=drefrep[:, blk * CB:(blk + 1) * CB],
                        op0=OP.mult, op1=OP.mult,
                        accum_out=sc2[:, st * NCB + blk:st * NCB + blk + 1],
                    )
                s1 = const.tile([P, 1], F32, tag=f"s1_{st}")
                s2 = const.tile([P, 1], F32, tag=f"s2_{st}")
                l1 = const.tile([P, 1], F32, tag=f"l1_{st}")
                l2 = const.tile([P, 1], F32, tag=f"l2_{st}")
                q = const.tile([P, 1], F32, tag=f"q_{st}")
                r = const.tile([P, 1], F32, tag=f"r_{st}")
                nc.vector.tensor_reduce(
                    s1[:], sc1[:, st * NCB:(st + 1) * NCB],
                    axis=mybir.AxisListType.X, op=OP.add,
                )
                nc.vector.tensor_reduce(
                    s2[:], sc2[:, st * NCB:(st + 1) * NCB],
                    axis=mybir.AxisListType.X, op=OP.add,
                )
                nc.scalar.activation(l1[:], s1[:], AF.Ln)
                nc.scalar.activation(l2[:], s2[:], AF.Ln)
                # r = exp(-0.5*(t*lnS1 + lnS2))
                nc.vector.scalar_tensor_tensor(
                    out=q[:], in0=l1[:], scalar=t, in1=l2[:],
                    op0=OP.mult, op1=OP.add,
                )
                nc.scalar.activation(r[:], q[:], AF.Exp, scale=-0.5)
                for blk in range(NCB):
                    o = opool.tile([P, CB], BF16, tag="o")
                    nc.vector.scalar_tensor_tensor(
                        out=o[:], in0=tstripe[:, blk * CB:(blk + 1) * CB],
                        scalar=r[:], in1=dinvrep[:, blk * CB:(blk + 1) * CB],
                        op0=OP.mult, op1=OP.mult,
                    )
                    nc.sync.dma_start(
                        out=w_out[st * P:(st + 1) * P, blk * CB:(blk + 1) * CB],
                        in_=o[:],
                    )

    _install_wait_split(nc)
    return nc


def _prep_inputs(X, Xref, s):
    """Host-side O((N+M)*D) prep of the augmented bf16 operand tables."""
    X = np.asarray(X, dtype=np.float32)
    Xref = np.asarray(Xref, dtype=np.float32)
    s = np.float32(s)

    # moving-side table: b = 2s * xref, plus -s*||xref||^2 rows
    b = (2.0 * s) * Xref.T                      # (16, M)
    bh, bl = _hilo(b)
    bn = -(s * np.sum(Xref * Xref, axis=1))     # (M,)
    bnh, bnl = _hilo(bn)
    rtab = np.zeros((2 * D + 2, M), dtype=ml_dtypes.bfloat16)
    rtab[0:D] = bh
    rtab[D:2 * D] = bl
    rtab[2 * D] = bnh
    rtab[2 * D + 1] = bnl

    def lhs_table(A):
        a = A.T                                  # (16, rows)
        ah, al = _hilo(a)
        tab = np.ones((2 * D + 2, A.shape[0]), dtype=ml_dtypes.bfloat16)
        tab[0:D] = ah
        tab[D:2 * D] = al
        return tab

    def bias_table(A):
        v = -(s * np.sum(A * A, axis=1))         # (rows,)
        return np.ascontiguousarray(v.reshape(NST, P).T)   # (P, NST)

    return rtab, lhs_table, bias_table


_prog_cache = {}


def kernel(X, Xref, log_eps, log_t):
    X = np.asarray(X, dtype=np.float32)
    Xref = np.asarray(Xref, dtype=np.float32)
    eps = _softplus(np.float32(log_eps))
    t = _softplus(np.float32(log_t))
    s = np.float32(1.0 / (4.0 * eps))

    key = (float(t),)
    if key not in _prog_cache:
        _prog_cache[key] = _build_program(t)
    nc = _prog_cache[key]

    rtab, lhs_table, bias_table = _prep_inputs(X, Xref, s)

    in_maps = []
    for k in range(NCORES):
        xs = X[k * SH:(k + 1) * SH]
        rs = Xref[k * SH:(k + 1) * SH]
        in_maps.append({
            "rtab": rtab,
            "lx": lhs_table(xs),
            "lr": lhs_table(rs),
            "bx": bias_table(xs),
            "br": bias_table(rs),
        })

    res = run_bass_kernel_spmd(nc, in_maps, list(range(NCORES)))
    global _last_results
    _last_results = res
    out = np.empty((N, M), dtype=np.float32)
    for k in range(NCORES):
        out[k * SH:(k + 1) * SH] = res.results[k]["out"]
    return out


_last_results = None

